# revision 17
# baseline (speedup 1.0000x reference)
"""NeoformerMHA Trainium2 kernel.

Math (per batch b):
  xt = x[b].T                          # [T, F]
  Q = xt@Wq.T+bq; K,V similar          # [T, F] -> heads [T, 4, 128]
  rope on Q,K (feature quarters, sin/cos tables, t-dependent)
  S1 = Q1 K1^T/16, S2 = Q2 K2^T/16     # per head, [T, T]
  A = softmax(S1) - s2[h]*softmax(S2)
  R = A V ; out = (xt + R).T           # [F, T]

Sharding: 8 cores = (b in 0..3) x (head-pair j in 0..1). Each core gets
x[b] [512, 2048], W*/b* row-slices [256, 512]/[256], s2 pair, and
produces out[b, j*256:(j+1)*256, :].

On-core layout (all f32, matmuls tagged float32r):
  QT,KT  [128=hf, T] per head (features on partitions)
  V      [T, 256] (time on partitions, 16 tiles of [128, 256])
  ST     [k, q] scores transposed -> exp on ScalarE (PSUM->SBUF, N=2048,
         S1|S2 packed side by side), no max-subtraction (|S|/16 < ~3.5)
  sums over k (partitions) via running DVE add over kc, then an all-ones
  [128,128] matmul that both reduces and broadcasts l across partitions;
  reciprocal on DVE; normalization applied to RT after the PV matmul.
  RT     [128=hf, q]  = V[kc]-as-lhsT @ PT  accumulated over kc in PSUM.
"""

import numpy as np

B, F, T, H = 4, 512, 2048, 4
HF = F // H          # 128  head features
CPC = F // 2         # 256  channels per core (2 heads)
NKC = T // 128       # 16   k chunks
QB = 1024            # q block
NQB = T // QB        # 2
W4 = HF // 4         # 32   rope quarter width

_CACHE = {}


def _rope_tables():
    # match reference: U = arange(T)*2^-j/16 ; Ur=sin, Ui=cos; f32 throughout
    pt = np.arange(T, dtype=np.float32)[None, :]
    df = (2.0 ** (-np.arange(W4, dtype=np.float32)))[:, None]
    u = pt * df / np.float32(16.0)
    ur = np.sin(u).astype(np.float32)   # [32, T]
    ui = np.cos(u).astype(np.float32)
    # dst = src*A + swap(src)*B, swap = r/i 32-row block exchange:
    #  r rows: src_r*Ur + swap(=i)*(-Ui) = r*Ur - i*Ui
    #  i rows: src_i*Ur + swap(=r)*(+Ui) = r*Ui + i*Ur
    ua = np.concatenate([ur, ur, ur, ur], axis=0)
    ub = np.concatenate([-ui, ui, -ui, ui], axis=0)
    return ua, ub


def _build_nc():
    import contextlib

    import concourse.mybir as mybir
    import concourse.tile as tile
    from concourse import bacc

    dt = mybir.dt
    f32 = dt.float32
    f32r = dt.float32r

    nc = bacc.Bacc("TRN2", target_bir_lowering=False)

    x_d = nc.dram_tensor("x_b", [F, T], f32, kind="ExternalInput")
    wq_d = nc.dram_tensor("wq", [CPC, F], f32, kind="ExternalInput")
    wk_d = nc.dram_tensor("wk", [CPC, F], f32, kind="ExternalInput")
    wv_d = nc.dram_tensor("wv", [CPC, F], f32, kind="ExternalInput")
    bq_d = nc.dram_tensor("bq", [CPC], f32, kind="ExternalInput")
    bk_d = nc.dram_tensor("bk", [CPC], f32, kind="ExternalInput")
    bv_d = nc.dram_tensor("bv", [CPC], f32, kind="ExternalInput")
    s2_d = nc.dram_tensor("s2h", [2], f32, kind="ExternalInput")
    xr_d = nc.dram_tensor("x_res", [CPC, T], f32, kind="ExternalInput")
    out_d = nc.dram_tensor("out_c", [CPC, T], f32, kind="ExternalOutput")

    uan, ubn = _rope_tables()
    ua_d = nc.inline_tensor(uan, "ua_t")
    ub_d = nc.inline_tensor(ubn, "ub_t")
    id_d = nc.inline_tensor(np.eye(128, dtype=np.float32), "ident")

    EXP = mybir.ActivationFunctionType.Exp

    with tile.TileContext(nc) as tc:
        with contextlib.ExitStack() as ctx:
            mem = ctx.enter_context(tc.tile_pool(name="mem", bufs=1))
            ps_s = ctx.enter_context(tc.tile_pool(name="psS", bufs=1, space="PSUM"))
            ps_r = ctx.enter_context(tc.tile_pool(name="psR", bufs=2, space="PSUM"))

            def mtile(shape, tag, bufs=1, d=None):
                return mem.tile(shape, d or f32, tag=tag, name=tag, bufs=bufs)

            # ---- load constants / inputs ----
            x_sb = []
            for fc in range(4):
                stg = mtile([128, T], "scr", bufs=3)
                nc.sync.dma_start(out=stg, in_=x_d[fc * 128:(fc + 1) * 128, :])
                xt_ = mtile([128, T], f"x{fc}", d=f32r)
                nc.vector.tensor_copy(xt_, stg)
                x_sb.append(xt_)
            xr_sb = []
            for mc in range(2):
                xt_ = mtile([128, T], f"xr{mc}")
                nc.sync.dma_start(out=xt_, in_=xr_d[mc * 128:(mc + 1) * 128, :])
                xr_sb.append(xt_)
            ua_sb = mtile([128, T], "ua")  # rows: Ur, Ui, Ur, Ui (32 each)
            ub_sb = mtile([128, T], "ub")  # rows: -Ui, Ur, -Ui, Ur
            nc.sync.dma_start(out=ua_sb, in_=ua_d[:, :])
            nc.sync.dma_start(out=ub_sb, in_=ub_d[:, :])
            id_sb = mtile([128, 128], "id")
            nc.sync.dma_start(out=id_sb, in_=id_d[:, :])
            ones_f = mtile([128, 128], "onesf")
            nc.vector.memset(ones_f, 1.0)
            ones_sb = mtile([128, 128], "ones", d=f32r)
            nc.vector.tensor_copy(ones_sb, ones_f)

            def bias_tiles(bd, tag):
                ts_ = []
                b2 = bd.rearrange("(c o) -> c o", o=1)
                for mc in range(2):
                    t_ = mtile([128, 1], f"{tag}{mc}")
                    nc.sync.dma_start(out=t_, in_=b2[mc * 128:(mc + 1) * 128, :])
                    ts_.append(t_)
                return ts_

            bq_t = bias_tiles(bq_d, "bq")
            bk_t = bias_tiles(bk_d, "bk")
            bv_t = bias_tiles(bv_d, "bv")
            s2_t = []
            for mc in range(2):
                t_ = mtile([128, 1], f"s2{mc}")
                nc.sync.dma_start(out=t_, in_=s2_d[mc:mc + 1].to_broadcast((128, 1)))
                s2_t.append(t_)
            # effective V-bias: bv*(1-s2) added at the end
            vbe_t = []
            one1 = mtile([128, 1], "one1")
            nc.vector.memset(one1, 1.0)
            for mc in range(2):
                t_ = mtile([128, 1], f"vbe{mc}")
                nc.vector.tensor_sub(t_, one1, s2_t[mc])
                nc.vector.tensor_mul(t_, t_, bv_t[mc])
                vbe_t.append(t_)

            # ---- W transposes: [256,512] -> 4 tiles [128f, 256c] ----
            def transpose_w(w_d, tag):
                wt = [mtile([128, CPC], f"{tag}T{fb}", d=f32r) for fb in range(4)]
                for ct in range(2):
                    raw = mtile([128, F], "wr", bufs=2)
                    nc.sync.dma_start(out=raw, in_=w_d[ct * 128:(ct + 1) * 128, :])
                    for fb in range(4):
                        pst = ps_r.tile([128, 128], f32, tag="R", name="psT")
                        nc.tensor.transpose(pst, raw[:, fb * 128:(fb + 1) * 128], id_sb)
                        nc.vector.tensor_copy(wt[fb][:, ct * 128:(ct + 1) * 128], pst)
                return wt

            wqT = transpose_w(wq_d, "wq")
            wkT = transpose_w(wk_d, "wk")
            wvT = transpose_w(wv_d, "wv")

            # ---- projections ----
            # QT/KT: [128c(head), T] = (W slice) @ x
            def project_qk(wT, b_t):
                res = []
                for mc in range(2):
                    ps = ps_s.tile([128, T], f32, tag="S", name="psQK")
                    for fc in range(4):
                        lhs = wT[fc][:, mc * 128:(mc + 1) * 128]
                        for ntc in range(4):
                            nc.tensor.matmul(
                                ps[:, ntc * 512:(ntc + 1) * 512],
                                lhs, x_sb[fc][:, ntc * 512:(ntc + 1) * 512],
                                start=(fc == 0), stop=(fc == 3))
                    sb = mtile([128, T], "qk", bufs=2, d=f32r)
                    nc.vector.tensor_scalar_add(sb, ps, b_t[mc])
                    res.append(sb)
                return res

            qt = project_qk(wqT, bq_t)
            kt = project_qk(wkT, bk_t)

            # V: [128t, 256c] x 16
            v_sb = []
            for tc_ in range(NKC):
                ps = ps_r.tile([128, CPC], f32, tag="R", name="psV")
                for fc in range(4):
                    nc.tensor.matmul(
                        ps, x_sb[fc][:, tc_ * 128:(tc_ + 1) * 128], wvT[fc],
                        start=(fc == 0), stop=(fc == 3))
                vt = mtile([128, CPC], f"v{tc_}", d=f32r)
                nc.vector.tensor_copy(vt, ps)
                v_sb.append(vt)

            # ---- rope ----
            # src rows [0:32]=g1r [32:64]=g1i [64:96]=g2r [96:128]=g2i
            # out_r = r*Ur - i*Ui ; out_i = r*Ui + i*Ur   (Ur=sin, Ui=cos)
            # => dst = src*A + swap(src)*B with the packed A/B tables, where
            # swap exchanges the r/i 32-row blocks (done via SBUF->SBUF DMA
            # because compute engines need same-start-partition operands).
            rq, rk = [], []
            for mc in range(2):
                for nm, src, dst_list in (("q", qt[mc], rq), ("k", kt[mc], rk)):
                    dst = mtile([128, T], f"r{nm}{mc}", d=f32r)
                    swp = mtile([128, T], "scr2", d=f32r)
                    tmp = mtile([128, T], "scr", bufs=3)
                    for g in range(2):
                        rr = slice(g * 64, g * 64 + 32)
                        ri = slice(g * 64 + 32, g * 64 + 64)
                        nc.sync.dma_start(out=swp[rr], in_=src[ri])
                        nc.sync.dma_start(out=swp[ri], in_=src[rr])
                    nc.vector.tensor_mul(tmp, src, ua_sb)
                    nc.vector.tensor_mul(dst, swp, ub_sb)
                    nc.vector.tensor_add(dst, dst, tmp)
                    dst_list.append(dst)

            # ---- attention per head / q-block ----
            for mc in range(2):
                vsl = [v_sb[kc][:, mc * 128:(mc + 1) * 128] for kc in range(NKC)]
                for qb in range(NQB):
                    q0 = qb * QB
                    r1 = ps_r.tile([128, QB], f32, tag="R", name="psR1")
                    r2 = ps_r.tile([128, QB], f32, tag="R", name="psR2")
                    sump = mtile([128, 2 * QB], "x2", d=f32r)
                    for kc in range(NKC):
                        s12 = ps_s.tile([128, 2 * QB], f32, tag="S", name="psS12")
                        for g in range(2):
                            gr = slice(g * 64, (g + 1) * 64)
                            lhs = rk[mc][gr, kc * 128:(kc + 1) * 128]
                            for hq in range(QB // 512):
                                nc.tensor.matmul(
                                    s12[:, g * QB + hq * 512: g * QB + hq * 512 + 512],
                                    lhs,
                                    rq[mc][gr, q0 + hq * 512: q0 + hq * 512 + 512],
                                    start=True, stop=True)
                        p12 = mtile([128, 2 * QB], f"x{kc % 2}", d=f32r)
                        nc.scalar.activation(p12, s12, EXP, scale=1.0 / 16.0)
                        if kc == 0:
                            nc.vector.tensor_copy(sump, p12)
                        else:
                            nc.vector.tensor_add(sump, sump, p12)
                        for g, racc in ((0, r1), (1, r2)):
                            for hq in range(QB // 512):
                                nc.tensor.matmul(
                                    racc[:, hq * 512:hq * 512 + 512],
                                    vsl[kc],
                                    p12[:, g * QB + hq * 512: g * QB + hq * 512 + 512],
                                    start=(kc == 0), stop=(kc == NKC - 1))
                    # reduce+broadcast row-sums over partitions: ones^T @ sumP
                    l12 = ps_s.tile([128, 2 * QB], f32, tag="S", name="psL12")
                    for hq in range(2 * QB // 512):
                        nc.tensor.matmul(
                            l12[:, hq * 512:(hq + 1) * 512],
                            ones_sb, sump[:, hq * 512:(hq + 1) * 512],
                            start=True, stop=True)
                    w12 = mtile([128, 2 * QB], "x3")
                    nc.vector.reciprocal(w12, l12)
                    nc.vector.tensor_scalar_mul(w12[:, QB:], w12[:, QB:], s2_t[mc])
                    y1 = mtile([128, QB], "scr", bufs=3)
                    y2 = mtile([128, QB], "scr", bufs=3)
                    nc.vector.tensor_mul(y1, r1, w12[:, :QB])
                    nc.vector.tensor_mul(y2, r2, w12[:, QB:])
                    ot = mtile([128, QB], "out", bufs=2)
                    nc.vector.tensor_sub(ot, y1, y2)
                    nc.vector.tensor_add(ot, ot, xr_sb[mc][:, q0:q0 + QB])
                    nc.vector.tensor_scalar_add(ot, ot, vbe_t[mc])
                    nc.sync.dma_start(
                        out=out_d[mc * 128:(mc + 1) * 128, q0:q0 + QB], in_=ot)

    return nc


def _get_nc():
    if "nc" not in _CACHE:
        nc = _build_nc()
        nc.compile()
        _CACHE["nc"] = nc
    return _CACHE["nc"]


def kernel(**inputs):
    from concourse.bass_utils import run_bass_kernel_spmd

    x = np.ascontiguousarray(np.asarray(inputs["x"], dtype=np.float32))
    wq = np.asarray(inputs["Wq"], dtype=np.float32)
    wk = np.asarray(inputs["Wk"], dtype=np.float32)
    wv = np.asarray(inputs["Wv"], dtype=np.float32)
    bq = np.asarray(inputs["bq"], dtype=np.float32)
    bk = np.asarray(inputs["bk"], dtype=np.float32)
    bv = np.asarray(inputs["bv"], dtype=np.float32)
    s2 = np.asarray(inputs["s2"], dtype=np.float32)

    nc = _get_nc()
    in_maps = []
    for core in range(8):
        b, j = core // 2, core % 2
        sl = slice(j * CPC, (j + 1) * CPC)
        in_maps.append({
            "x_b": np.ascontiguousarray(x[b]),
            "wq": np.ascontiguousarray(wq[sl]),
            "wk": np.ascontiguousarray(wk[sl]),
            "wv": np.ascontiguousarray(wv[sl]),
            "bq": np.ascontiguousarray(bq[sl]),
            "bk": np.ascontiguousarray(bk[sl]),
            "bv": np.ascontiguousarray(bv[sl]),
            "s2h": np.ascontiguousarray(s2[2 * j:2 * j + 2]),
            "x_res": np.ascontiguousarray(x[b, sl, :]),
        })
    res = run_bass_kernel_spmd(nc, in_maps, core_ids=list(range(8)))
    out = np.empty((B, F, T), dtype=np.float32)
    for core in range(8):
        b, j = core // 2, core % 2
        out[b, j * CPC:(j + 1) * CPC, :] = res.results[core]["out_c"]
    return out


# revision 23
# speedup vs baseline: 2.0331x; 2.0331x over previous
"""NeoformerMHA Trainium2 kernel.

Math (per batch b):
  xt = x[b].T                          # [T, F]
  Q = xt@Wq.T+bq; K,V similar          # [T, F] -> heads [T, 4, 128]
  rope on Q,K (feature quarters, sin/cos tables, t-dependent)
  S1 = Q1 K1^T/16, S2 = Q2 K2^T/16     # per head, [T, T]
  A = softmax(S1) - s2[h]*softmax(S2)
  R = A V ; out = (xt + R).T           # [F, T]

Sharding: 8 cores = (b in 0..3) x (head-pair j in 0..1). Each core gets
x[b] [512, 2048], W*/b* row-slices [256, 512]/[256], s2 pair, and
produces out[b, j*256:(j+1)*256, :].

On-core layout (all f32, matmuls tagged float32r):
  QT,KT  [128=hf, T] per head (features on partitions)
  V      [T, 256] (time on partitions, 16 tiles of [128, 256])
  ST     [k, q] scores transposed -> exp on ScalarE (PSUM->SBUF, N=2048,
         S1|S2 packed side by side), no max-subtraction (|S|/16 < ~3.5)
  sums over k (partitions) via running DVE add over kc, then an all-ones
  [128,128] matmul that both reduces and broadcasts l across partitions;
  reciprocal on DVE; normalization applied to RT after the PV matmul.
  RT     [128=hf, q]  = V[kc]-as-lhsT @ PT  accumulated over kc in PSUM.
"""

import numpy as np

B, F, T, H = 4, 512, 2048, 4
HF = F // H          # 128  head features
CPC = F // 2         # 256  channels per core (2 heads)
NKC = T // 128       # 16   k chunks
QB = 512             # q block
NQB = T // QB        # 4
L_MODE = "dve16"       # softmax-denominator accumulation: "pe" | "dve16"
W4 = HF // 4         # 32   rope quarter width

_CACHE = {}


def _rope_tables():
    # match reference: U = arange(T)*2^-j/16 ; Ur=sin, Ui=cos; f32 throughout
    pt = np.arange(T, dtype=np.float32)[None, :]
    df = (2.0 ** (-np.arange(W4, dtype=np.float32)))[:, None]
    u = pt * df / np.float32(16.0)
    ur = np.sin(u).astype(np.float32)   # [32, T]
    ui = np.cos(u).astype(np.float32)
    # dst = src*A + swap(src)*B, swap = r/i 32-row block exchange:
    #  r rows: src_r*Ur + swap(=i)*(-Ui) = r*Ur - i*Ui
    #  i rows: src_i*Ur + swap(=r)*(+Ui) = r*Ui + i*Ur
    ua = np.concatenate([ur, ur, ur, ur], axis=0)
    ub = np.concatenate([-ui, ui, -ui, ui], axis=0)
    return ua, ub


def _build_nc():
    import contextlib

    import concourse.mybir as mybir
    import concourse.tile as tile
    from concourse import bacc

    dt = mybir.dt
    f32 = dt.float32
    f32r = dt.float32r

    nc = bacc.Bacc("TRN2", target_bir_lowering=False)

    x_d = nc.dram_tensor("x_b", [F, T], f32, kind="ExternalInput")
    wq_d = nc.dram_tensor("wq", [CPC, F], f32, kind="ExternalInput")
    wk_d = nc.dram_tensor("wk", [CPC, F], f32, kind="ExternalInput")
    wv_d = nc.dram_tensor("wv", [CPC, F], f32, kind="ExternalInput")
    bq_d = nc.dram_tensor("bq", [CPC], f32, kind="ExternalInput")
    bk_d = nc.dram_tensor("bk", [CPC], f32, kind="ExternalInput")
    bv_d = nc.dram_tensor("bv", [CPC], f32, kind="ExternalInput")
    s2_d = nc.dram_tensor("s2h", [2], f32, kind="ExternalInput")
    xr_d = nc.dram_tensor("x_res", [CPC, T], f32, kind="ExternalInput")
    out_d = nc.dram_tensor("out_c", [CPC, T], f32, kind="ExternalOutput")

    uan, ubn = _rope_tables()
    ua_d = nc.inline_tensor(uan, "ua_t")
    ub_d = nc.inline_tensor(ubn, "ub_t")
    id_d = nc.inline_tensor(np.eye(128, dtype=np.float32), "ident")

    EXP = mybir.ActivationFunctionType.Exp

    with tile.TileContext(nc) as tc:
        with contextlib.ExitStack() as ctx:
            mem = ctx.enter_context(tc.tile_pool(name="mem", bufs=1))
            ps_s = ctx.enter_context(tc.tile_pool(name="psS", bufs=2, space="PSUM"))
            ps_r = ctx.enter_context(tc.tile_pool(name="psR", bufs=2, space="PSUM"))
            ps_l = ctx.enter_context(tc.tile_pool(name="psL", bufs=1, space="PSUM"))

            def mtile(shape, tag, bufs=1, d=None):
                return mem.tile(shape, d or f32, tag=tag, name=tag, bufs=bufs)

            # ---- load constants / inputs (W + identity first: they gate
            # the PE transposes; x next; rope tables / residual last) ----
            id_sb = mtile([128, 128], "id")
            nc.sync.dma_start(out=id_sb, in_=id_d[:, :])
            wraw = {}
            for wnm, w_d in (("q", wq_d), ("k", wk_d), ("v", wv_d)):
                for ct in range(2):
                    rw = mtile([128, F], f"wr{wnm}{ct}")
                    nc.sync.dma_start(out=rw, in_=w_d[ct * 128:(ct + 1) * 128, :])
                    wraw[(wnm, ct)] = rw
            x_sb = []
            for fc in range(4):
                stg = mtile([128, T], "scr", bufs=3)
                nc.sync.dma_start(out=stg, in_=x_d[fc * 128:(fc + 1) * 128, :])
                xt_ = mtile([128, T], f"x{fc}", d=f32r)
                nc.vector.tensor_copy(xt_, stg)
                x_sb.append(xt_)
            ua_sb = mtile([128, T], "ua")  # rows: Ur (x4 blocks)
            ub_sb = mtile([128, T], "ub")  # rows: -Ui, Ui, -Ui, Ui
            nc.sync.dma_start(out=ua_sb, in_=ua_d[:, :])
            nc.sync.dma_start(out=ub_sb, in_=ub_d[:, :])
            xr_sb = []
            for mc in range(2):
                xt_ = mtile([128, T], f"xr{mc}")
                nc.sync.dma_start(out=xt_, in_=xr_d[mc * 128:(mc + 1) * 128, :])
                xr_sb.append(xt_)
            ones_f = mtile([128, 128], "onesf")
            nc.vector.memset(ones_f, 1.0)
            P_DT = f32r if L_MODE == "pe" else dt.bfloat16
            ones_sb = mtile([128, 128], "ones", d=P_DT)
            nc.vector.tensor_copy(ones_sb, ones_f)

            def bias_tiles(bd, tag):
                ts_ = []
                b2 = bd.rearrange("(c o) -> c o", o=1)
                for mc in range(2):
                    t_ = mtile([128, 1], f"{tag}{mc}")
                    nc.sync.dma_start(out=t_, in_=b2[mc * 128:(mc + 1) * 128, :])
                    ts_.append(t_)
                return ts_

            bq_t = bias_tiles(bq_d, "bq")
            bk_t = bias_tiles(bk_d, "bk")
            bv_t = bias_tiles(bv_d, "bv")
            s2_t = []
            for mc in range(2):
                t_ = mtile([128, 1], f"s2{mc}")
                nc.sync.dma_start(out=t_, in_=s2_d[mc:mc + 1].to_broadcast((128, 1)))
                s2_t.append(t_)
            # effective V-bias: bv*(1-s2) added at the end
            vbe_t = []
            one1 = mtile([128, 1], "one1")
            nc.vector.memset(one1, 1.0)
            for mc in range(2):
                t_ = mtile([128, 1], f"vbe{mc}")
                nc.vector.tensor_sub(t_, one1, s2_t[mc])
                nc.vector.tensor_mul(t_, t_, bv_t[mc])
                vbe_t.append(t_)

            # ---- W transposes: [256,512] -> 4 tiles [128f, 256c] ----
            def transpose_w(wnm):
                wt = [mtile([128, CPC], f"w{wnm}T{fb}", d=f32r) for fb in range(4)]
                for ct in range(2):
                    raw = wraw[(wnm, ct)]
                    for fb in range(4):
                        pst = ps_r.tile([128, 128], f32, tag="R", name="psT")
                        nc.tensor.transpose(pst, raw[:, fb * 128:(fb + 1) * 128], id_sb)
                        nc.vector.tensor_copy(wt[fb][:, ct * 128:(ct + 1) * 128], pst)
                return wt

            wqT = transpose_w("q")
            wkT = transpose_w("k")
            wvT = transpose_w("v")

            # ---- projections ----
            # QT/KT: [128c(head), T] = (W slice) @ x
            def project_qk1(wT, b_t, mc):
                sb = mtile([128, T], "qk", bufs=2, d=f32r)
                for th in range(2):
                    t0 = th * (T // 2)
                    ps = ps_s.tile([128, T // 2], f32, tag="S", name="psQK")
                    for fc in range(4):
                        lhs = wT[fc][:, mc * 128:(mc + 1) * 128]
                        for ntc in range(2):
                            nc.tensor.matmul(
                                ps[:, ntc * 512:(ntc + 1) * 512],
                                lhs,
                                x_sb[fc][:, t0 + ntc * 512:t0 + (ntc + 1) * 512],
                                start=(fc == 0), stop=(fc == 3))
                    nc.vector.tensor_scalar_add(
                        sb[:, t0:t0 + T // 2], ps, b_t[mc])
                return sb


            # rope: src rows [0:32]=g1r [32:64]=g1i [64:96]=g2r [96:128]=g2i
            # out_r = r*Ur - i*Ui ; out_i = r*Ui + i*Ur   (Ur=sin, Ui=cos)
            # => dst = src*A + swap(src)*B with the packed A/B tables, where
            # swap exchanges the r/i 32-row blocks (done via SBUF->SBUF DMA
            # because compute engines need same-start-partition operands).
            def rope(nm, mc, srct):
                dst = mtile([128, T], f"r{nm}{mc}", d=f32r)
                swp = mtile([128, T], "scr2", bufs=2, d=f32r)
                tmp = mtile([128, T], "scr", bufs=3)
                for g in range(2):
                    rr = slice(g * 64, g * 64 + 32)
                    ri = slice(g * 64 + 32, g * 64 + 64)
                    nc.sync.dma_start(out=swp[rr], in_=srct[ri])
                    nc.sync.dma_start(out=swp[ri], in_=srct[rr])
                nc.vector.tensor_mul(tmp, srct, ua_sb)
                nc.vector.tensor_mul(dst, swp, ub_sb)
                nc.vector.tensor_add(dst, dst, tmp)
                return dst

            rq, rk = [], []
            # head 0 projections + rope first so attention can start early;
            # V projection + head 1 fill the PE while head 0 rope runs on DVE
            rq.append(None); rk.append(None)  # placeholders
            q0t = project_qk1(wqT, bq_t, 0)
            k0t = project_qk1(wkT, bk_t, 0)
            rq[0] = rope("q", 0, q0t)
            rk[0] = rope("k", 0, k0t)
            q1t = project_qk1(wqT, bq_t, 1)
            k1t = project_qk1(wkT, bk_t, 1)

            # V: [128t, 256c] x 16
            V_DT = f32r if L_MODE == "pe" else dt.bfloat16
            v_sb = []
            for tc_ in range(NKC):
                ps = ps_r.tile([128, CPC], f32, tag="R", name="psV")
                for fc in range(4):
                    nc.tensor.matmul(
                        ps, x_sb[fc][:, tc_ * 128:(tc_ + 1) * 128], wvT[fc],
                        start=(fc == 0), stop=(fc == 3))
                vt = mtile([128, CPC], f"v{tc_}", d=V_DT)
                nc.vector.tensor_copy(vt, ps)
                v_sb.append(vt)

            rq.append(rope("q", 1, q1t))
            rk.append(rope("k", 1, k1t))

            # ---- attention per head / q-block ----
            # s12/p12 layout per kc: [128, 1024] = S1(512 q) | S2(512 q)
            for mc in range(2):
                vsl = [v_sb[kc][:, mc * 128:(mc + 1) * 128] for kc in range(NKC)]
                for qb in range(NQB):
                    q0 = qb * QB
                    r1 = ps_r.tile([128, QB], f32, tag="R", name="psR1")
                    r2 = ps_r.tile([128, QB], f32, tag="R", name="psR2")
                    l12 = ps_l.tile([128, 2 * QB], f32, tag="L", name="psL12")
                    if L_MODE == "dve16":
                        sump = mtile([128, 2 * QB], "x2", d=dt.bfloat16)
                    for kc in range(NKC):
                        s12 = ps_s.tile([128, 2 * QB], f32, tag="S", name="psS12")
                        for g in range(2):
                            gr = slice(g * 64, (g + 1) * 64)
                            nc.tensor.matmul(
                                s12[:, g * QB:(g + 1) * QB],
                                rk[mc][gr, kc * 128:(kc + 1) * 128],
                                rq[mc][gr, q0:q0 + QB],
                                start=True, stop=True)
                        p12 = mtile([128, 2 * QB], f"x{kc % 2}", d=P_DT)
                        nc.scalar.activation(p12, s12, EXP, scale=1.0 / 16.0)
                        if L_MODE == "pe":
                            for g in range(2):
                                nc.tensor.matmul(
                                    l12[:, g * QB:(g + 1) * QB],
                                    ones_sb, p12[:, g * QB:(g + 1) * QB],
                                    start=(kc == 0), stop=(kc == NKC - 1))
                        else:
                            if kc == 0:
                                nc.vector.tensor_copy(sump, p12)
                            else:
                                nc.vector.tensor_add(sump, sump, p12)
                        for g, racc in ((0, r1), (1, r2)):
                            nc.tensor.matmul(
                                racc,
                                vsl[kc],
                                p12[:, g * QB:(g + 1) * QB],
                                start=(kc == 0), stop=(kc == NKC - 1))
                    if L_MODE == "dve16":
                        for g in range(2):
                            nc.tensor.matmul(
                                l12[:, g * QB:(g + 1) * QB],
                                ones_sb, sump[:, g * QB:(g + 1) * QB],
                                start=True, stop=True)
                    w12 = mtile([128, 2 * QB], "x3")
                    nc.vector.reciprocal(w12, l12)
                    nc.vector.tensor_scalar_mul(w12[:, QB:], w12[:, QB:], s2_t[mc])
                    y1 = mtile([128, QB], "scr", bufs=3)
                    y2 = mtile([128, QB], "scr", bufs=3)
                    nc.vector.tensor_mul(y1, r1, w12[:, :QB])
                    nc.vector.tensor_mul(y2, r2, w12[:, QB:])
                    ot = mtile([128, QB], "out", bufs=2)
                    nc.vector.tensor_sub(ot, y1, y2)
                    nc.vector.tensor_add(ot, ot, xr_sb[mc][:, q0:q0 + QB])
                    nc.vector.tensor_scalar_add(ot, ot, vbe_t[mc])
                    nc.sync.dma_start(
                        out=out_d[mc * 128:(mc + 1) * 128, q0:q0 + QB], in_=ot)

    return nc


def _get_nc():
    if "nc" not in _CACHE:
        nc = _build_nc()
        nc.compile()
        _CACHE["nc"] = nc
    return _CACHE["nc"]


def kernel(**inputs):
    from concourse.bass_utils import run_bass_kernel_spmd

    x = np.ascontiguousarray(np.asarray(inputs["x"], dtype=np.float32))
    wq = np.asarray(inputs["Wq"], dtype=np.float32)
    wk = np.asarray(inputs["Wk"], dtype=np.float32)
    wv = np.asarray(inputs["Wv"], dtype=np.float32)
    bq = np.asarray(inputs["bq"], dtype=np.float32)
    bk = np.asarray(inputs["bk"], dtype=np.float32)
    bv = np.asarray(inputs["bv"], dtype=np.float32)
    s2 = np.asarray(inputs["s2"], dtype=np.float32)

    nc = _get_nc()
    in_maps = []
    for core in range(8):
        b, j = core // 2, core % 2
        sl = slice(j * CPC, (j + 1) * CPC)
        in_maps.append({
            "x_b": np.ascontiguousarray(x[b]),
            "wq": np.ascontiguousarray(wq[sl]),
            "wk": np.ascontiguousarray(wk[sl]),
            "wv": np.ascontiguousarray(wv[sl]),
            "bq": np.ascontiguousarray(bq[sl]),
            "bk": np.ascontiguousarray(bk[sl]),
            "bv": np.ascontiguousarray(bv[sl]),
            "s2h": np.ascontiguousarray(s2[2 * j:2 * j + 2]),
            "x_res": np.ascontiguousarray(x[b, sl, :]),
        })
    res = run_bass_kernel_spmd(nc, in_maps, core_ids=list(range(8)))
    out = np.empty((B, F, T), dtype=np.float32)
    for core in range(8):
        b, j = core // 2, core % 2
        out[b, j * CPC:(j + 1) * CPC, :] = res.results[core]["out_c"]
    return out


# revision 27
# speedup vs baseline: 4.2197x; 2.0755x over previous
"""NeoformerMHA Trainium2 kernel.

Math (per batch b):
  xt = x[b].T                          # [T, F]
  Q = xt@Wq.T+bq; K,V similar          # [T, F] -> heads [T, 4, 128]
  rope on Q,K (feature quarters, sin/cos tables, t-dependent)
  S1 = Q1 K1^T/16, S2 = Q2 K2^T/16     # per head, [T, T]
  A = softmax(S1) - s2[h]*softmax(S2)
  R = A V ; out = (xt + R).T           # [F, T]

Sharding: 8 cores = (b in 0..3) x (head-pair j in 0..1). Each core gets
x[b] [512, 2048], W*/b* row-slices [256, 512]/[256], s2 pair, and
produces out[b, j*256:(j+1)*256, :].

On-core layout (all f32, matmuls tagged float32r):
  QT,KT  [128=hf, T] per head (features on partitions)
  V      [T, 256] (time on partitions, 16 tiles of [128, 256])
  ST     [k, q] scores transposed -> exp on ScalarE (PSUM->SBUF, N=2048,
         S1|S2 packed side by side), no max-subtraction (|S|/16 < ~3.5)
  sums over k (partitions) via running DVE add over kc, then an all-ones
  [128,128] matmul that both reduces and broadcasts l across partitions;
  reciprocal on DVE; normalization applied to RT after the PV matmul.
  RT     [128=hf, q]  = V[kc]-as-lhsT @ PT  accumulated over kc in PSUM.
"""

import numpy as np

B, F, T, H = 4, 512, 2048, 4
HF = F // H          # 128  head features
CPC = F // 2         # 256  channels per core (2 heads)
NKC = T // 128       # 16   k chunks
QB = 512             # q block
NQB = T // QB        # 4
L_MODE = "dve16"       # softmax-denominator accumulation: "pe" | "dve16"
W4 = HF // 4         # 32   rope quarter width

_CACHE = {}


def _rope_tables():
    # match reference: U = arange(T)*2^-j/16 ; Ur=sin, Ui=cos; f32 throughout
    pt = np.arange(T, dtype=np.float32)[None, :]
    df = (2.0 ** (-np.arange(W4, dtype=np.float32)))[:, None]
    u = pt * df / np.float32(16.0)
    ur = np.sin(u).astype(np.float32)   # [32, T]
    ui = np.cos(u).astype(np.float32)
    # dst = src*A + swap(src)*B, swap = r/i 32-row block exchange:
    #  r rows: src_r*Ur + swap(=i)*(-Ui) = r*Ur - i*Ui
    #  i rows: src_i*Ur + swap(=r)*(+Ui) = r*Ui + i*Ur
    ua = np.concatenate([ur, ur, ur, ur], axis=0)
    ub = np.concatenate([-ui, ui, -ui, ui], axis=0)
    return ua, ub


def _build_nc():
    import contextlib

    import concourse.mybir as mybir
    import concourse.tile as tile
    from concourse import bacc

    dt = mybir.dt
    f32 = dt.float32
    f32r = dt.float32r

    nc = bacc.Bacc("TRN2", target_bir_lowering=False)

    x_d = nc.dram_tensor("x_b", [F, T], f32, kind="ExternalInput")
    wq_d = nc.dram_tensor("wq", [CPC, F], f32, kind="ExternalInput")
    wk_d = nc.dram_tensor("wk", [CPC, F], f32, kind="ExternalInput")
    wv_d = nc.dram_tensor("wv", [CPC, F], f32, kind="ExternalInput")
    bq_d = nc.dram_tensor("bq", [CPC], f32, kind="ExternalInput")
    bk_d = nc.dram_tensor("bk", [CPC], f32, kind="ExternalInput")
    bv_d = nc.dram_tensor("bv", [CPC], f32, kind="ExternalInput")
    s2_d = nc.dram_tensor("s2h", [2], f32, kind="ExternalInput")
    xr_d = nc.dram_tensor("x_res", [CPC, T], f32, kind="ExternalInput")
    out_d = nc.dram_tensor("out_c", [CPC, T], f32, kind="ExternalOutput")

    uan, ubn = _rope_tables()
    ua_d = nc.inline_tensor(uan, "ua_t")
    ub_d = nc.inline_tensor(ubn, "ub_t")
    id_d = nc.inline_tensor(np.eye(128, dtype=np.float32), "ident")

    EXP = mybir.ActivationFunctionType.Exp

    with tile.TileContext(nc) as tc:
        with contextlib.ExitStack() as ctx:
            mem = ctx.enter_context(tc.tile_pool(name="mem", bufs=1))
            ps_s = ctx.enter_context(tc.tile_pool(name="psS", bufs=2, space="PSUM"))
            ps_r = ctx.enter_context(tc.tile_pool(name="psR", bufs=2, space="PSUM"))
            ps_l = ctx.enter_context(tc.tile_pool(name="psL", bufs=1, space="PSUM"))

            def mtile(shape, tag, bufs=1, d=None):
                return mem.tile(shape, d or f32, tag=tag, name=tag, bufs=bufs)

            # ---- load constants / inputs (W + identity first: they gate
            # the PE transposes; x next; rope tables / residual last) ----
            id_sb = mtile([128, 128], "id")
            nc.sync.dma_start(out=id_sb, in_=id_d[:, :])
            wraw = {}
            for wnm, w_d in (("q", wq_d), ("k", wk_d), ("v", wv_d)):
                for ct in range(2):
                    rw = mtile([128, F], f"wr{wnm}{ct}")
                    nc.sync.dma_start(out=rw, in_=w_d[ct * 128:(ct + 1) * 128, :])
                    wraw[(wnm, ct)] = rw
            x_sb = []
            for fc in range(4):
                stg = mtile([128, T], "scr", bufs=3)
                nc.sync.dma_start(out=stg, in_=x_d[fc * 128:(fc + 1) * 128, :])
                xt_ = mtile([128, T], f"x{fc}", d=f32r)
                nc.vector.tensor_copy(xt_, stg)
                x_sb.append(xt_)
            ua_sb = mtile([128, T], "ua")  # rows: Ur (x4 blocks)
            ub_sb = mtile([128, T], "ub")  # rows: -Ui, Ui, -Ui, Ui
            nc.sync.dma_start(out=ua_sb, in_=ua_d[:, :])
            nc.sync.dma_start(out=ub_sb, in_=ub_d[:, :])
            xr_sb = []
            for mc in range(2):
                xt_ = mtile([128, T], f"xr{mc}")
                nc.sync.dma_start(out=xt_, in_=xr_d[mc * 128:(mc + 1) * 128, :])
                xr_sb.append(xt_)
            ones_f = mtile([128, 128], "onesf")
            nc.vector.memset(ones_f, 1.0)
            P_DT = f32r if L_MODE == "pe" else dt.bfloat16
            ones_sb = mtile([128, 128], "ones", d=P_DT)
            nc.vector.tensor_copy(ones_sb, ones_f)

            def bias_tiles(bd, tag):
                ts_ = []
                b2 = bd.rearrange("(c o) -> c o", o=1)
                for mc in range(2):
                    t_ = mtile([128, 1], f"{tag}{mc}")
                    nc.sync.dma_start(out=t_, in_=b2[mc * 128:(mc + 1) * 128, :])
                    ts_.append(t_)
                return ts_

            bq_t = bias_tiles(bq_d, "bq")
            bk_t = bias_tiles(bk_d, "bk")
            bv_t = bias_tiles(bv_d, "bv")
            s2_t = []
            for mc in range(2):
                t_ = mtile([128, 1], f"s2{mc}")
                nc.sync.dma_start(out=t_, in_=s2_d[mc:mc + 1].to_broadcast((128, 1)))
                s2_t.append(t_)
            # effective V-bias: bv*(1-s2) added at the end
            vbe_t = []
            one1 = mtile([128, 1], "one1")
            nc.vector.memset(one1, 1.0)
            for mc in range(2):
                t_ = mtile([128, 1], f"vbe{mc}")
                nc.vector.tensor_sub(t_, one1, s2_t[mc])
                nc.vector.tensor_mul(t_, t_, bv_t[mc])
                vbe_t.append(t_)

            # ---- W transposes: [256,512] -> 4 tiles [128f, 256c] ----
            def transpose_w(wnm):
                wt = [mtile([128, CPC], f"w{wnm}T{fb}", d=f32r) for fb in range(4)]
                for ct in range(2):
                    raw = wraw[(wnm, ct)]
                    for fb in range(4):
                        pst = ps_r.tile([128, 128], f32, tag="R", name="psT")
                        nc.tensor.transpose(pst, raw[:, fb * 128:(fb + 1) * 128], id_sb)
                        nc.vector.tensor_copy(wt[fb][:, ct * 128:(ct + 1) * 128], pst)
                return wt

            wqT = transpose_w("q")
            wkT = transpose_w("k")
            wvT = transpose_w("v")

            # ---- projections ----
            # QT/KT: [128c(head), T] = (W slice) @ x
            def project_qk1(wT, b_t, mc):
                sb = mtile([128, T], "qk", bufs=2, d=f32r)
                for th in range(2):
                    t0 = th * (T // 2)
                    ps = ps_s.tile([128, T // 2], f32, tag="S", name="psQK")
                    for fc in range(4):
                        lhs = wT[fc][:, mc * 128:(mc + 1) * 128]
                        for ntc in range(2):
                            nc.tensor.matmul(
                                ps[:, ntc * 512:(ntc + 1) * 512],
                                lhs,
                                x_sb[fc][:, t0 + ntc * 512:t0 + (ntc + 1) * 512],
                                start=(fc == 0), stop=(fc == 3))
                    nc.scalar.activation(
                        sb[:, t0:t0 + T // 2], ps,
                        mybir.ActivationFunctionType.Identity, bias=b_t[mc])
                return sb


            # rope: src rows [0:32]=g1r [32:64]=g1i [64:96]=g2r [96:128]=g2i
            # out_r = r*Ur - i*Ui ; out_i = r*Ui + i*Ur   (Ur=sin, Ui=cos)
            # => dst = src*A + swap(src)*B with the packed A/B tables, where
            # swap exchanges the r/i 32-row blocks (done via SBUF->SBUF DMA
            # because compute engines need same-start-partition operands).
            def rope(nm, mc, srct):
                dst = mtile([128, T], f"r{nm}{mc}", d=f32r)
                swp = mtile([128, T], "scr2", bufs=2, d=f32r)
                tmp = mtile([128, T], "scr", bufs=3)
                for g in range(2):
                    rr = slice(g * 64, g * 64 + 32)
                    ri = slice(g * 64 + 32, g * 64 + 64)
                    nc.sync.dma_start(out=swp[rr], in_=srct[ri])
                    nc.sync.dma_start(out=swp[ri], in_=srct[rr])
                nc.vector.tensor_mul(tmp, srct, ua_sb)
                nc.vector.tensor_mul(dst, swp, ub_sb)
                nc.vector.tensor_add(dst, dst, tmp)
                return dst

            rq, rk = [], []
            # head 0 projections + rope first so attention can start early;
            # V projection + head 1 fill the PE while head 0 rope runs on DVE
            rq.append(None); rk.append(None)  # placeholders
            q0t = project_qk1(wqT, bq_t, 0)
            k0t = project_qk1(wkT, bk_t, 0)
            rq[0] = rope("q", 0, q0t)
            rk[0] = rope("k", 0, k0t)
            # V: [128t, 256c] x 16  (before head-1 proj: attention mc0
            # needs V; head-1 proj then overlaps attention mc0)
            V_DT = f32r if L_MODE == "pe" else dt.bfloat16
            v_sb = []
            for tc_ in range(NKC):
                ps = ps_r.tile([128, CPC], f32, tag="R", name="psV")
                for fc in range(4):
                    nc.tensor.matmul(
                        ps, x_sb[fc][:, tc_ * 128:(tc_ + 1) * 128], wvT[fc],
                        start=(fc == 0), stop=(fc == 3))
                vt = mtile([128, CPC], f"v{tc_}", d=V_DT)
                nc.scalar.copy(vt, ps)
                v_sb.append(vt)

            q1t = project_qk1(wqT, bq_t, 1)
            k1t = project_qk1(wkT, bk_t, 1)
            rq.append(rope("q", 1, q1t))
            rk.append(rope("k", 1, k1t))

            # ---- attention per head / q-block ----
            # s12/p12 layout per kc: [128, 1024] = S1(512 q) | S2(512 q)
            for mc in range(2):
                vsl = [v_sb[kc][:, mc * 128:(mc + 1) * 128] for kc in range(NKC)]
                for qb in range(NQB):
                    q0 = qb * QB
                    r1 = ps_r.tile([128, QB], f32, tag="R", name="psR1")
                    r2 = ps_r.tile([128, QB], f32, tag="R", name="psR2")
                    l12 = ps_l.tile([128, 2 * QB], f32, tag="L", name="psL12")
                    if L_MODE == "dve16":
                        sump = mtile([128, 2 * QB], "x2", d=dt.bfloat16)
                    for kc in range(NKC):
                        s12 = ps_s.tile([128, 2 * QB], f32, tag="S", name="psS12")
                        for g in range(2):
                            gr = slice(g * 64, (g + 1) * 64)
                            nc.tensor.matmul(
                                s12[:, g * QB:(g + 1) * QB],
                                rk[mc][gr, kc * 128:(kc + 1) * 128],
                                rq[mc][gr, q0:q0 + QB],
                                start=True, stop=True)
                        p12 = mtile([128, 2 * QB], f"x{kc % 2}", d=P_DT)
                        nc.scalar.activation(p12, s12, EXP, scale=1.0 / 16.0)
                        if L_MODE == "pe":
                            for g in range(2):
                                nc.tensor.matmul(
                                    l12[:, g * QB:(g + 1) * QB],
                                    ones_sb, p12[:, g * QB:(g + 1) * QB],
                                    start=(kc == 0), stop=(kc == NKC - 1))
                        else:
                            if kc == 0:
                                nc.vector.tensor_copy(sump, p12)
                            else:
                                nc.vector.tensor_add(sump, sump, p12)
                        for g, racc in ((0, r1), (1, r2)):
                            nc.tensor.matmul(
                                racc,
                                vsl[kc],
                                p12[:, g * QB:(g + 1) * QB],
                                start=(kc == 0), stop=(kc == NKC - 1))
                    if L_MODE == "dve16":
                        for g in range(2):
                            nc.tensor.matmul(
                                l12[:, g * QB:(g + 1) * QB],
                                ones_sb, sump[:, g * QB:(g + 1) * QB],
                                start=True, stop=True)
                    w12 = mtile([128, 2 * QB], "x3")
                    nc.vector.reciprocal(w12, l12)
                    nc.vector.tensor_scalar_mul(w12[:, QB:], w12[:, QB:], s2_t[mc])
                    y1 = mtile([128, QB], "scr", bufs=3)
                    y2 = mtile([128, QB], "scr", bufs=3)
                    nc.vector.tensor_mul(y1, r1, w12[:, :QB])
                    nc.vector.tensor_mul(y2, r2, w12[:, QB:])
                    ot = mtile([128, QB], "out", bufs=2)
                    nc.vector.tensor_sub(ot, y1, y2)
                    nc.vector.tensor_add(ot, ot, xr_sb[mc][:, q0:q0 + QB])
                    nc.vector.tensor_scalar_add(ot, ot, vbe_t[mc])
                    nc.sync.dma_start(
                        out=out_d[mc * 128:(mc + 1) * 128, q0:q0 + QB], in_=ot)

    return nc


def _get_nc():
    if "nc" not in _CACHE:
        nc = _build_nc()
        nc.compile()
        _CACHE["nc"] = nc
    return _CACHE["nc"]


def kernel(**inputs):
    from concourse.bass_utils import run_bass_kernel_spmd

    x = np.ascontiguousarray(np.asarray(inputs["x"], dtype=np.float32))
    wq = np.asarray(inputs["Wq"], dtype=np.float32)
    wk = np.asarray(inputs["Wk"], dtype=np.float32)
    wv = np.asarray(inputs["Wv"], dtype=np.float32)
    bq = np.asarray(inputs["bq"], dtype=np.float32)
    bk = np.asarray(inputs["bk"], dtype=np.float32)
    bv = np.asarray(inputs["bv"], dtype=np.float32)
    s2 = np.asarray(inputs["s2"], dtype=np.float32)

    nc = _get_nc()
    in_maps = []
    for core in range(8):
        b, j = core // 2, core % 2
        sl = slice(j * CPC, (j + 1) * CPC)
        in_maps.append({
            "x_b": np.ascontiguousarray(x[b]),
            "wq": np.ascontiguousarray(wq[sl]),
            "wk": np.ascontiguousarray(wk[sl]),
            "wv": np.ascontiguousarray(wv[sl]),
            "bq": np.ascontiguousarray(bq[sl]),
            "bk": np.ascontiguousarray(bk[sl]),
            "bv": np.ascontiguousarray(bv[sl]),
            "s2h": np.ascontiguousarray(s2[2 * j:2 * j + 2]),
            "x_res": np.ascontiguousarray(x[b, sl, :]),
        })
    res = run_bass_kernel_spmd(nc, in_maps, core_ids=list(range(8)))
    out = np.empty((B, F, T), dtype=np.float32)
    for core in range(8):
        b, j = core // 2, core % 2
        out[b, j * CPC:(j + 1) * CPC, :] = res.results[core]["out_c"]
    return out


# revision 31
# speedup vs baseline: 5.2477x; 1.2436x over previous
"""NeoformerMHA Trainium2 kernel.

Math (per batch b):
  xt = x[b].T                          # [T, F]
  Q = xt@Wq.T+bq; K,V similar          # [T, F] -> heads [T, 4, 128]
  rope on Q,K (feature quarters, sin/cos tables, t-dependent)
  S1 = Q1 K1^T/16, S2 = Q2 K2^T/16     # per head, [T, T]
  A = softmax(S1) - s2[h]*softmax(S2)
  R = A V ; out = (xt + R).T           # [F, T]

Sharding: 8 cores = (b in 0..3) x (head-pair j in 0..1). Each core gets
x[b] [512, 2048], W*/b* row-slices [256, 512]/[256], s2 pair, and
produces out[b, j*256:(j+1)*256, :].

On-core layout (f32 activations; QKV projections + S-matmuls in
float32r = full PE rate; P = exp(S) and V in bf16 for the PV path):
  QT,KT  [128=hf, T] per head (features on partitions)
  V      [T, 256] (time on partitions, 16 tiles of [128, 256])
  ST     [k, q] scores transposed -> exp on ScalarE (PSUM->SBUF,
         S1|S2 packed side by side), no max-subtraction (|S|/16 < ~3.5)
  softmax denominators: running bf16 DVE add over kc, then an all-ones
  [128,128] matmul that both reduces and broadcasts l across partitions;
  reciprocal on DVE; normalization applied to RT after the PV matmul.
  RT     [128=hf, q]  = V[kc]-as-lhsT @ PT  accumulated over kc in PSUM.
Emission order pipelines phases: W/id DMAs first, head-0 proj+rope, V,
head-1 proj+rope overlap head-0 attention; projection evictions ride
the otherwise-idle ScalarE.
"""

import numpy as np

B, F, T, H = 4, 512, 2048, 4
HF = F // H          # 128  head features
CPC = F // 2         # 256  channels per core (2 heads)
NKC = T // 128       # 16   k chunks
QB = 512             # q block
NQB = T // QB        # 4
L_MODE = "dve16"       # softmax-denominator accumulation: "pe" | "dve16"
W4 = HF // 4         # 32   rope quarter width

_CACHE = {}


def _rope_tables():
    # match reference: U = arange(T)*2^-j/16 ; Ur=sin, Ui=cos; f32 throughout
    pt = np.arange(T, dtype=np.float32)[None, :]
    df = (2.0 ** (-np.arange(W4, dtype=np.float32)))[:, None]
    u = pt * df / np.float32(16.0)
    ur = np.sin(u).astype(np.float32)   # [32, T]
    ui = np.cos(u).astype(np.float32)
    # dst = src*A + swap(src)*B, swap = r/i 32-row block exchange:
    #  r rows: src_r*Ur + swap(=i)*(-Ui) = r*Ur - i*Ui
    #  i rows: src_i*Ur + swap(=r)*(+Ui) = r*Ui + i*Ur
    ua = np.concatenate([ur, ur, ur, ur], axis=0)
    ub = np.concatenate([-ui, ui, -ui, ui], axis=0)
    return ua, ub


def _build_nc():
    import contextlib

    import concourse.mybir as mybir
    import concourse.tile as tile
    from concourse import bacc

    dt = mybir.dt
    f32 = dt.float32
    f32r = dt.float32r

    nc = bacc.Bacc("TRN2", target_bir_lowering=False)

    x_d = nc.dram_tensor("x_b", [F, T], f32, kind="ExternalInput")
    wq_d = nc.dram_tensor("wq", [CPC, F], f32, kind="ExternalInput")
    wk_d = nc.dram_tensor("wk", [CPC, F], f32, kind="ExternalInput")
    wv_d = nc.dram_tensor("wv", [CPC, F], f32, kind="ExternalInput")
    bq_d = nc.dram_tensor("bq", [CPC], f32, kind="ExternalInput")
    bk_d = nc.dram_tensor("bk", [CPC], f32, kind="ExternalInput")
    bv_d = nc.dram_tensor("bv", [CPC], f32, kind="ExternalInput")
    s2_d = nc.dram_tensor("s2h", [2], f32, kind="ExternalInput")
    xr_d = nc.dram_tensor("x_res", [CPC, T], f32, kind="ExternalInput")
    out_d = nc.dram_tensor("out_c", [CPC, T], f32, kind="ExternalOutput")

    uan, ubn = _rope_tables()
    ua_d = nc.inline_tensor(uan, "ua_t")
    ub_d = nc.inline_tensor(ubn, "ub_t")
    id_d = nc.inline_tensor(np.eye(128, dtype=np.float32), "ident")

    EXP = mybir.ActivationFunctionType.Exp

    with tile.TileContext(nc) as tc:
        with contextlib.ExitStack() as ctx:
            mem = ctx.enter_context(tc.tile_pool(name="mem", bufs=1))
            ps_s = ctx.enter_context(tc.tile_pool(name="psS", bufs=2, space="PSUM"))
            ps_r = ctx.enter_context(tc.tile_pool(name="psR", bufs=2, space="PSUM"))
            ps_l = ctx.enter_context(tc.tile_pool(name="psL", bufs=1, space="PSUM"))

            def mtile(shape, tag, bufs=1, d=None):
                return mem.tile(shape, d or f32, tag=tag, name=tag, bufs=bufs)

            # ---- load constants / inputs (W + identity first: they gate
            # the PE transposes; x next; rope tables / residual last) ----
            id_sb = mtile([128, 128], "id")
            nc.sync.dma_start(out=id_sb, in_=id_d[:, :])
            wraw = {}
            for wnm, w_d in (("q", wq_d), ("k", wk_d), ("v", wv_d)):
                for ct in range(2):
                    rw = mtile([128, F], f"wr{wnm}{ct}")
                    nc.sync.dma_start(out=rw, in_=w_d[ct * 128:(ct + 1) * 128, :])
                    wraw[(wnm, ct)] = rw
            x_sb = []
            for fc in range(4):
                stg = mtile([128, T], "scr", bufs=3)
                xt_ = mtile([128, T], f"x{fc}", d=f32r)
                for th in range(2):
                    hs = slice(th * (T // 2), (th + 1) * (T // 2))
                    nc.sync.dma_start(out=stg[:, hs],
                                      in_=x_d[fc * 128:(fc + 1) * 128, hs])
                    nc.vector.tensor_copy(xt_[:, hs], stg[:, hs])
                x_sb.append(xt_)
            ua_sb = mtile([128, T], "ua")  # rows: Ur (x4 blocks)
            ub_sb = mtile([128, T], "ub")  # rows: -Ui, Ui, -Ui, Ui
            nc.sync.dma_start(out=ua_sb, in_=ua_d[:, :])
            nc.sync.dma_start(out=ub_sb, in_=ub_d[:, :])
            xr_sb = []
            for mc in range(2):
                xt_ = mtile([128, T], f"xr{mc}")
                nc.gpsimd.dma_start(out=xt_, in_=xr_d[mc * 128:(mc + 1) * 128, :])
                xr_sb.append(xt_)
            ones_f = mtile([128, 128], "onesf")
            nc.vector.memset(ones_f, 1.0)
            P_DT = f32r if L_MODE == "pe" else dt.bfloat16
            ones_sb = mtile([128, 128], "ones", d=P_DT)
            nc.vector.tensor_copy(ones_sb, ones_f)

            def bias_tiles(bd, tag):
                ts_ = []
                b2 = bd.rearrange("(c o) -> c o", o=1)
                for mc in range(2):
                    t_ = mtile([128, 1], f"{tag}{mc}")
                    nc.sync.dma_start(out=t_, in_=b2[mc * 128:(mc + 1) * 128, :])
                    ts_.append(t_)
                return ts_

            bq_t = bias_tiles(bq_d, "bq")
            bk_t = bias_tiles(bk_d, "bk")
            bv_t = bias_tiles(bv_d, "bv")
            s2_t = []
            for mc in range(2):
                t_ = mtile([128, 1], f"s2{mc}")
                nc.sync.dma_start(out=t_, in_=s2_d[mc:mc + 1].to_broadcast((128, 1)))
                s2_t.append(t_)
            # effective V-bias: bv*(1-s2) added at the end
            vbe_t = []
            one1 = mtile([128, 1], "one1")
            nc.vector.memset(one1, 1.0)
            for mc in range(2):
                t_ = mtile([128, 1], f"vbe{mc}")
                nc.vector.tensor_sub(t_, one1, s2_t[mc])
                nc.vector.tensor_mul(t_, t_, bv_t[mc])
                vbe_t.append(t_)

            # ---- W transposes: [256,512] -> 4 tiles [128f, 256c] ----
            def transpose_w(wnm):
                wt = [mtile([128, CPC], f"w{wnm}T{fb}", d=f32r) for fb in range(4)]
                for ct in range(2):
                    raw = wraw[(wnm, ct)]
                    for fb in range(4):
                        pst = ps_r.tile([128, 128], f32, tag="R", name="psT")
                        nc.tensor.transpose(pst, raw[:, fb * 128:(fb + 1) * 128], id_sb)
                        nc.vector.tensor_copy(wt[fb][:, ct * 128:(ct + 1) * 128], pst)
                return wt

            wqT = transpose_w("q")
            wkT = transpose_w("k")
            wvT = transpose_w("v")

            # ---- projections ----
            # QT/KT: [128c(head), T] = (W slice) @ x
            def project_qk1(wT, b_t, mc):
                sb = mtile([128, T], "qk", bufs=2, d=f32r)
                for th in range(2):
                    t0 = th * (T // 2)
                    ps = ps_s.tile([128, T // 2], f32, tag="S", name="psQK")
                    for fc in range(4):
                        lhs = wT[fc][:, mc * 128:(mc + 1) * 128]
                        for ntc in range(2):
                            nc.tensor.matmul(
                                ps[:, ntc * 512:(ntc + 1) * 512],
                                lhs,
                                x_sb[fc][:, t0 + ntc * 512:t0 + (ntc + 1) * 512],
                                start=(fc == 0), stop=(fc == 3))
                    nc.scalar.activation(
                        sb[:, t0:t0 + T // 2], ps,
                        mybir.ActivationFunctionType.Identity, bias=b_t[mc])
                return sb


            # rope: src rows [0:32]=g1r [32:64]=g1i [64:96]=g2r [96:128]=g2i
            # out_r = r*Ur - i*Ui ; out_i = r*Ui + i*Ur   (Ur=sin, Ui=cos)
            # => dst = src*A + swap(src)*B with the packed A/B tables, where
            # swap exchanges the r/i 32-row blocks (done via SBUF->SBUF DMA
            # because compute engines need same-start-partition operands).
            def rope(nm, mc, srct):
                dst = mtile([128, T], f"r{nm}{mc}", d=f32r)
                swp = mtile([128, T], "scr2", bufs=2, d=f32r)
                tmp = mtile([128, T], "scr", bufs=3)
                for g in range(2):
                    rr = slice(g * 64, g * 64 + 32)
                    ri = slice(g * 64 + 32, g * 64 + 64)
                    nc.sync.dma_start(out=swp[rr], in_=srct[ri])
                    nc.sync.dma_start(out=swp[ri], in_=srct[rr])
                nc.vector.tensor_mul(tmp, srct, ua_sb)
                nc.vector.tensor_mul(dst, swp, ub_sb)
                nc.vector.tensor_add(dst, dst, tmp)
                return dst

            rq, rk = [], []
            # head 0 projections + rope first so attention can start early;
            # V projection + head 1 fill the PE while head 0 rope runs on DVE
            rq.append(None); rk.append(None)  # placeholders
            q0t = project_qk1(wqT, bq_t, 0)
            k0t = project_qk1(wkT, bk_t, 0)
            rq[0] = rope("q", 0, q0t)
            rk[0] = rope("k", 0, k0t)
            # V: [128t, 256c] x 16  (before head-1 proj: attention mc0
            # needs V; head-1 proj then overlaps attention mc0)
            V_DT = f32r if L_MODE == "pe" else dt.bfloat16
            v_sb = []
            for tc_ in range(NKC):
                ps = ps_r.tile([128, CPC], f32, tag="R", name="psV")
                for fc in range(4):
                    nc.tensor.matmul(
                        ps, x_sb[fc][:, tc_ * 128:(tc_ + 1) * 128], wvT[fc],
                        start=(fc == 0), stop=(fc == 3))
                vt = mtile([128, CPC], f"v{tc_}", d=V_DT)
                nc.scalar.copy(vt, ps)
                v_sb.append(vt)

            q1t = project_qk1(wqT, bq_t, 1)
            k1t = project_qk1(wkT, bk_t, 1)
            rq.append(rope("q", 1, q1t))
            rk.append(rope("k", 1, k1t))

            # ---- attention per head / q-block ----
            # s12/p12 layout per kc: [128, 1024] = S1(512 q) | S2(512 q)
            for mc in range(2):
                vsl = [v_sb[kc][:, mc * 128:(mc + 1) * 128] for kc in range(NKC)]
                for qb in range(NQB):
                    q0 = qb * QB
                    r1 = ps_r.tile([128, QB], f32, tag="R", name="psR1")
                    r2 = ps_r.tile([128, QB], f32, tag="R", name="psR2")
                    l12 = ps_l.tile([128, 2 * QB], f32, tag="L", name="psL12")
                    if L_MODE == "dve16":
                        sump = mtile([128, 2 * QB], "x2", d=dt.bfloat16)
                    for kc in range(NKC):
                        s12 = ps_s.tile([128, 2 * QB], f32, tag="S", name="psS12")
                        for g in range(2):
                            gr = slice(g * 64, (g + 1) * 64)
                            nc.tensor.matmul(
                                s12[:, g * QB:(g + 1) * QB],
                                rk[mc][gr, kc * 128:(kc + 1) * 128],
                                rq[mc][gr, q0:q0 + QB],
                                start=True, stop=True)
                        p12 = mtile([128, 2 * QB], f"x{kc % 2}", d=P_DT)
                        nc.scalar.activation(p12, s12, EXP, scale=1.0 / 16.0)
                        if L_MODE == "pe":
                            for g in range(2):
                                nc.tensor.matmul(
                                    l12[:, g * QB:(g + 1) * QB],
                                    ones_sb, p12[:, g * QB:(g + 1) * QB],
                                    start=(kc == 0), stop=(kc == NKC - 1))
                        else:
                            if kc == 0:
                                nc.vector.tensor_copy(sump, p12)
                            else:
                                nc.vector.tensor_add(sump, sump, p12)
                        for g, racc in ((0, r1), (1, r2)):
                            nc.tensor.matmul(
                                racc,
                                vsl[kc],
                                p12[:, g * QB:(g + 1) * QB],
                                start=(kc == 0), stop=(kc == NKC - 1))
                    if L_MODE == "dve16":
                        for g in range(2):
                            nc.tensor.matmul(
                                l12[:, g * QB:(g + 1) * QB],
                                ones_sb, sump[:, g * QB:(g + 1) * QB],
                                start=True, stop=True)
                    w12 = mtile([128, 2 * QB], "x3")
                    nc.vector.reciprocal(w12, l12)
                    nc.vector.tensor_scalar_mul(w12[:, QB:], w12[:, QB:], s2_t[mc])
                    y1 = mtile([128, QB], "scr", bufs=3)
                    y2 = mtile([128, QB], "scr", bufs=3)
                    nc.vector.tensor_mul(y1, r1, w12[:, :QB])
                    nc.vector.tensor_mul(y2, r2, w12[:, QB:])
                    ot = mtile([128, QB], "out", bufs=2)
                    nc.vector.tensor_sub(ot, y1, y2)
                    nc.vector.tensor_add(ot, ot, xr_sb[mc][:, q0:q0 + QB])
                    nc.vector.tensor_scalar_add(ot, ot, vbe_t[mc])
                    nc.sync.dma_start(
                        out=out_d[mc * 128:(mc + 1) * 128, q0:q0 + QB], in_=ot)

    return nc


def _get_nc():
    if "nc" not in _CACHE:
        nc = _build_nc()
        nc.compile()
        _CACHE["nc"] = nc
    return _CACHE["nc"]


def kernel(**inputs):
    from concourse.bass_utils import run_bass_kernel_spmd

    x = np.ascontiguousarray(np.asarray(inputs["x"], dtype=np.float32))
    wq = np.asarray(inputs["Wq"], dtype=np.float32)
    wk = np.asarray(inputs["Wk"], dtype=np.float32)
    wv = np.asarray(inputs["Wv"], dtype=np.float32)
    bq = np.asarray(inputs["bq"], dtype=np.float32)
    bk = np.asarray(inputs["bk"], dtype=np.float32)
    bv = np.asarray(inputs["bv"], dtype=np.float32)
    s2 = np.asarray(inputs["s2"], dtype=np.float32)

    nc = _get_nc()
    in_maps = []
    for core in range(8):
        b, j = core // 2, core % 2
        sl = slice(j * CPC, (j + 1) * CPC)
        in_maps.append({
            "x_b": np.ascontiguousarray(x[b]),
            "wq": np.ascontiguousarray(wq[sl]),
            "wk": np.ascontiguousarray(wk[sl]),
            "wv": np.ascontiguousarray(wv[sl]),
            "bq": np.ascontiguousarray(bq[sl]),
            "bk": np.ascontiguousarray(bk[sl]),
            "bv": np.ascontiguousarray(bv[sl]),
            "s2h": np.ascontiguousarray(s2[2 * j:2 * j + 2]),
            "x_res": np.ascontiguousarray(x[b, sl, :]),
        })
    res = run_bass_kernel_spmd(nc, in_maps, core_ids=list(range(8)))
    out = np.empty((B, F, T), dtype=np.float32)
    for core in range(8):
        b, j = core // 2, core % 2
        out[b, j * CPC:(j + 1) * CPC, :] = res.results[core]["out_c"]
    return out


# revision 32
# speedup vs baseline: 5.4498x; 1.0385x over previous
"""NeoformerMHA Trainium2 kernel.

Math (per batch b):
  xt = x[b].T                          # [T, F]
  Q = xt@Wq.T+bq; K,V similar          # [T, F] -> heads [T, 4, 128]
  rope on Q,K (feature quarters, sin/cos tables, t-dependent)
  S1 = Q1 K1^T/16, S2 = Q2 K2^T/16     # per head, [T, T]
  A = softmax(S1) - s2[h]*softmax(S2)
  R = A V ; out = (xt + R).T           # [F, T]

Sharding: 8 cores = (b in 0..3) x (head-pair j in 0..1). Each core gets
x[b] [512, 2048], W*/b* row-slices [256, 512]/[256], s2 pair, and
produces out[b, j*256:(j+1)*256, :].

On-core layout (f32 activations; QKV projections + S-matmuls in
float32r = full PE rate; P = exp(S) and V in bf16 for the PV path):
  QT,KT  [128=hf, T] per head (features on partitions)
  V      [T, 256] (time on partitions, 16 tiles of [128, 256])
  ST     [k, q] scores transposed -> exp on ScalarE (PSUM->SBUF,
         S1|S2 packed side by side), no max-subtraction (|S|/16 < ~3.5)
  softmax denominators: running bf16 DVE add over kc, then an all-ones
  [128,128] matmul that both reduces and broadcasts l across partitions;
  reciprocal on DVE; normalization applied to RT after the PV matmul.
  RT     [128=hf, q]  = V[kc]-as-lhsT @ PT  accumulated over kc in PSUM.
Emission order pipelines phases: W/id DMAs first, head-0 proj+rope, V,
head-1 proj+rope overlap head-0 attention; projection evictions ride
the otherwise-idle ScalarE.
"""

import numpy as np

B, F, T, H = 4, 512, 2048, 4
HF = F // H          # 128  head features
CPC = F // 2         # 256  channels per core (2 heads)
NKC = T // 128       # 16   k chunks
QB = 512             # q block
NQB = T // QB        # 4
L_MODE = "pe"          # softmax-denominator accumulation: "pe" | "dve16"
W4 = HF // 4         # 32   rope quarter width

_CACHE = {}


def _rope_tables():
    # match reference: U = arange(T)*2^-j/16 ; Ur=sin, Ui=cos; f32 throughout
    pt = np.arange(T, dtype=np.float32)[None, :]
    df = (2.0 ** (-np.arange(W4, dtype=np.float32)))[:, None]
    u = pt * df / np.float32(16.0)
    ur = np.sin(u).astype(np.float32)   # [32, T]
    ui = np.cos(u).astype(np.float32)
    # dst = src*A + swap(src)*B, swap = r/i 32-row block exchange:
    #  r rows: src_r*Ur + swap(=i)*(-Ui) = r*Ur - i*Ui
    #  i rows: src_i*Ur + swap(=r)*(+Ui) = r*Ui + i*Ur
    ua = np.concatenate([ur, ur, ur, ur], axis=0)
    ub = np.concatenate([-ui, ui, -ui, ui], axis=0)
    return ua, ub


def _build_nc():
    import contextlib

    import concourse.mybir as mybir
    import concourse.tile as tile
    from concourse import bacc

    dt = mybir.dt
    f32 = dt.float32
    f32r = dt.float32r

    nc = bacc.Bacc("TRN2", target_bir_lowering=False)

    x_d = nc.dram_tensor("x_b", [F, T], f32, kind="ExternalInput")
    wq_d = nc.dram_tensor("wq", [CPC, F], f32, kind="ExternalInput")
    wk_d = nc.dram_tensor("wk", [CPC, F], f32, kind="ExternalInput")
    wv_d = nc.dram_tensor("wv", [CPC, F], f32, kind="ExternalInput")
    bq_d = nc.dram_tensor("bq", [CPC], f32, kind="ExternalInput")
    bk_d = nc.dram_tensor("bk", [CPC], f32, kind="ExternalInput")
    bv_d = nc.dram_tensor("bv", [CPC], f32, kind="ExternalInput")
    s2_d = nc.dram_tensor("s2h", [2], f32, kind="ExternalInput")
    xr_d = nc.dram_tensor("x_res", [CPC, T], f32, kind="ExternalInput")
    out_d = nc.dram_tensor("out_c", [CPC, T], f32, kind="ExternalOutput")

    uan, ubn = _rope_tables()
    ua_d = nc.inline_tensor(uan, "ua_t")
    ub_d = nc.inline_tensor(ubn, "ub_t")
    id_d = nc.inline_tensor(np.eye(128, dtype=np.float32), "ident")

    EXP = mybir.ActivationFunctionType.Exp

    with tile.TileContext(nc) as tc:
        with contextlib.ExitStack() as ctx:
            mem = ctx.enter_context(tc.tile_pool(name="mem", bufs=1))
            ps_s = ctx.enter_context(tc.tile_pool(name="psS", bufs=2, space="PSUM"))
            ps_r = ctx.enter_context(tc.tile_pool(name="psR", bufs=2, space="PSUM"))
            ps_l = ctx.enter_context(tc.tile_pool(name="psL", bufs=1, space="PSUM"))

            def mtile(shape, tag, bufs=1, d=None):
                return mem.tile(shape, d or f32, tag=tag, name=tag, bufs=bufs)

            # ---- load constants / inputs (W + identity first: they gate
            # the PE transposes; x next; rope tables / residual last) ----
            id_sb = mtile([128, 128], "id")
            nc.sync.dma_start(out=id_sb, in_=id_d[:, :])
            wraw = {}
            for wnm, w_d in (("q", wq_d), ("k", wk_d), ("v", wv_d)):
                for ct in range(2):
                    rw = mtile([128, F], f"wr{wnm}{ct}")
                    nc.sync.dma_start(out=rw, in_=w_d[ct * 128:(ct + 1) * 128, :])
                    wraw[(wnm, ct)] = rw
            x_sb = []
            for fc in range(4):
                stg = mtile([128, T], "scr", bufs=3)
                xt_ = mtile([128, T], f"x{fc}", d=f32r)
                for th in range(2):
                    hs = slice(th * (T // 2), (th + 1) * (T // 2))
                    nc.sync.dma_start(out=stg[:, hs],
                                      in_=x_d[fc * 128:(fc + 1) * 128, hs])
                    nc.vector.tensor_copy(xt_[:, hs], stg[:, hs])
                x_sb.append(xt_)
            ua_sb = mtile([128, T], "ua")  # rows: Ur (x4 blocks)
            ub_sb = mtile([128, T], "ub")  # rows: -Ui, Ui, -Ui, Ui
            nc.sync.dma_start(out=ua_sb, in_=ua_d[:, :])
            nc.sync.dma_start(out=ub_sb, in_=ub_d[:, :])
            xr_sb = []
            for mc in range(2):
                xt_ = mtile([128, T], f"xr{mc}")
                nc.gpsimd.dma_start(out=xt_, in_=xr_d[mc * 128:(mc + 1) * 128, :])
                xr_sb.append(xt_)
            ones_f = mtile([128, 128], "onesf")
            nc.vector.memset(ones_f, 1.0)
            P_DT = f32r if L_MODE == "pe" else dt.bfloat16
            ones_sb = mtile([128, 128], "ones", d=P_DT)
            nc.vector.tensor_copy(ones_sb, ones_f)

            def bias_tiles(bd, tag):
                ts_ = []
                b2 = bd.rearrange("(c o) -> c o", o=1)
                for mc in range(2):
                    t_ = mtile([128, 1], f"{tag}{mc}")
                    nc.sync.dma_start(out=t_, in_=b2[mc * 128:(mc + 1) * 128, :])
                    ts_.append(t_)
                return ts_

            bq_t = bias_tiles(bq_d, "bq")
            bk_t = bias_tiles(bk_d, "bk")
            bv_t = bias_tiles(bv_d, "bv")
            s2_t = []
            for mc in range(2):
                t_ = mtile([128, 1], f"s2{mc}")
                nc.sync.dma_start(out=t_, in_=s2_d[mc:mc + 1].to_broadcast((128, 1)))
                s2_t.append(t_)
            # effective V-bias: bv*(1-s2) added at the end
            vbe_t = []
            one1 = mtile([128, 1], "one1")
            nc.vector.memset(one1, 1.0)
            for mc in range(2):
                t_ = mtile([128, 1], f"vbe{mc}")
                nc.vector.tensor_sub(t_, one1, s2_t[mc])
                nc.vector.tensor_mul(t_, t_, bv_t[mc])
                vbe_t.append(t_)

            # ---- W transposes: [256,512] -> 4 tiles [128f, 256c] ----
            def transpose_w(wnm):
                wt = [mtile([128, CPC], f"w{wnm}T{fb}", d=f32r) for fb in range(4)]
                for ct in range(2):
                    raw = wraw[(wnm, ct)]
                    for fb in range(4):
                        pst = ps_r.tile([128, 128], f32, tag="R", name="psT")
                        nc.tensor.transpose(pst, raw[:, fb * 128:(fb + 1) * 128], id_sb)
                        nc.vector.tensor_copy(wt[fb][:, ct * 128:(ct + 1) * 128], pst)
                return wt

            wqT = transpose_w("q")
            wkT = transpose_w("k")
            wvT = transpose_w("v")

            # ---- projections ----
            # QT/KT: [128c(head), T] = (W slice) @ x
            def project_qk1(wT, b_t, mc):
                sb = mtile([128, T], "qk", bufs=2, d=f32r)
                for th in range(2):
                    t0 = th * (T // 2)
                    ps = ps_s.tile([128, T // 2], f32, tag="S", name="psQK")
                    for fc in range(4):
                        lhs = wT[fc][:, mc * 128:(mc + 1) * 128]
                        for ntc in range(2):
                            nc.tensor.matmul(
                                ps[:, ntc * 512:(ntc + 1) * 512],
                                lhs,
                                x_sb[fc][:, t0 + ntc * 512:t0 + (ntc + 1) * 512],
                                start=(fc == 0), stop=(fc == 3))
                    nc.scalar.activation(
                        sb[:, t0:t0 + T // 2], ps,
                        mybir.ActivationFunctionType.Identity, bias=b_t[mc])
                return sb


            # rope: src rows [0:32]=g1r [32:64]=g1i [64:96]=g2r [96:128]=g2i
            # out_r = r*Ur - i*Ui ; out_i = r*Ui + i*Ur   (Ur=sin, Ui=cos)
            # => dst = src*A + swap(src)*B with the packed A/B tables, where
            # swap exchanges the r/i 32-row blocks (done via SBUF->SBUF DMA
            # because compute engines need same-start-partition operands).
            def rope(nm, mc, srct):
                dst = mtile([128, T], f"r{nm}{mc}", d=f32r)
                swp = mtile([128, T], "scr2", bufs=2, d=f32r)
                tmp = mtile([128, T], "scr", bufs=3)
                for g in range(2):
                    rr = slice(g * 64, g * 64 + 32)
                    ri = slice(g * 64 + 32, g * 64 + 64)
                    nc.sync.dma_start(out=swp[rr], in_=srct[ri])
                    nc.sync.dma_start(out=swp[ri], in_=srct[rr])
                nc.vector.tensor_mul(tmp, srct, ua_sb)
                nc.vector.tensor_mul(dst, swp, ub_sb)
                nc.vector.tensor_add(dst, dst, tmp)
                return dst

            rq, rk = [], []
            # head 0 projections + rope first so attention can start early;
            # V projection + head 1 fill the PE while head 0 rope runs on DVE
            rq.append(None); rk.append(None)  # placeholders
            q0t = project_qk1(wqT, bq_t, 0)
            k0t = project_qk1(wkT, bk_t, 0)
            rq[0] = rope("q", 0, q0t)
            rk[0] = rope("k", 0, k0t)
            # V: [128t, 256c] x 16  (before head-1 proj: attention mc0
            # needs V; head-1 proj then overlaps attention mc0)
            V_DT = f32r if L_MODE == "pe" else dt.bfloat16
            v_sb = []
            for tc_ in range(NKC):
                ps = ps_r.tile([128, CPC], f32, tag="R", name="psV")
                for fc in range(4):
                    nc.tensor.matmul(
                        ps, x_sb[fc][:, tc_ * 128:(tc_ + 1) * 128], wvT[fc],
                        start=(fc == 0), stop=(fc == 3))
                vt = mtile([128, CPC], f"v{tc_}", d=V_DT)
                nc.scalar.copy(vt, ps)
                v_sb.append(vt)

            q1t = project_qk1(wqT, bq_t, 1)
            k1t = project_qk1(wkT, bk_t, 1)
            rq.append(rope("q", 1, q1t))
            rk.append(rope("k", 1, k1t))

            # ---- attention per head / q-block ----
            # s12/p12 layout per kc: [128, 1024] = S1(512 q) | S2(512 q)
            for mc in range(2):
                vsl = [v_sb[kc][:, mc * 128:(mc + 1) * 128] for kc in range(NKC)]
                for qb in range(NQB):
                    q0 = qb * QB
                    r1 = ps_r.tile([128, QB], f32, tag="R", name="psR1")
                    r2 = ps_r.tile([128, QB], f32, tag="R", name="psR2")
                    l12 = ps_l.tile([128, 2 * QB], f32, tag="L", name="psL12")
                    if L_MODE == "dve16":
                        sump = mtile([128, 2 * QB], "x2", d=dt.bfloat16)
                    for kc in range(NKC):
                        s12 = ps_s.tile([128, 2 * QB], f32, tag="S", name="psS12")
                        for g in range(2):
                            gr = slice(g * 64, (g + 1) * 64)
                            nc.tensor.matmul(
                                s12[:, g * QB:(g + 1) * QB],
                                rk[mc][gr, kc * 128:(kc + 1) * 128],
                                rq[mc][gr, q0:q0 + QB],
                                start=True, stop=True)
                        p12 = mtile([128, 2 * QB], f"x{kc % 2}", d=P_DT)
                        nc.scalar.activation(p12, s12, EXP, scale=1.0 / 16.0)
                        if L_MODE == "pe":
                            for g in range(2):
                                nc.tensor.matmul(
                                    l12[:, g * QB:(g + 1) * QB],
                                    ones_sb, p12[:, g * QB:(g + 1) * QB],
                                    start=(kc == 0), stop=(kc == NKC - 1))
                        else:
                            if kc == 0:
                                nc.vector.tensor_copy(sump, p12)
                            else:
                                nc.vector.tensor_add(sump, sump, p12)
                        for g, racc in ((0, r1), (1, r2)):
                            nc.tensor.matmul(
                                racc,
                                vsl[kc],
                                p12[:, g * QB:(g + 1) * QB],
                                start=(kc == 0), stop=(kc == NKC - 1))
                    if L_MODE == "dve16":
                        for g in range(2):
                            nc.tensor.matmul(
                                l12[:, g * QB:(g + 1) * QB],
                                ones_sb, sump[:, g * QB:(g + 1) * QB],
                                start=True, stop=True)
                    w12 = mtile([128, 2 * QB], "x3")
                    nc.vector.reciprocal(w12, l12)
                    nc.vector.tensor_scalar_mul(w12[:, QB:], w12[:, QB:], s2_t[mc])
                    y1 = mtile([128, QB], "scr", bufs=3)
                    y2 = mtile([128, QB], "scr", bufs=3)
                    nc.vector.tensor_mul(y1, r1, w12[:, :QB])
                    nc.vector.tensor_mul(y2, r2, w12[:, QB:])
                    ot = mtile([128, QB], "out", bufs=2)
                    nc.vector.tensor_sub(ot, y1, y2)
                    nc.vector.tensor_add(ot, ot, xr_sb[mc][:, q0:q0 + QB])
                    nc.vector.tensor_scalar_add(ot, ot, vbe_t[mc])
                    nc.sync.dma_start(
                        out=out_d[mc * 128:(mc + 1) * 128, q0:q0 + QB], in_=ot)

    return nc


def _get_nc():
    if "nc" not in _CACHE:
        nc = _build_nc()
        nc.compile()
        _CACHE["nc"] = nc
    return _CACHE["nc"]


def kernel(**inputs):
    from concourse.bass_utils import run_bass_kernel_spmd

    x = np.ascontiguousarray(np.asarray(inputs["x"], dtype=np.float32))
    wq = np.asarray(inputs["Wq"], dtype=np.float32)
    wk = np.asarray(inputs["Wk"], dtype=np.float32)
    wv = np.asarray(inputs["Wv"], dtype=np.float32)
    bq = np.asarray(inputs["bq"], dtype=np.float32)
    bk = np.asarray(inputs["bk"], dtype=np.float32)
    bv = np.asarray(inputs["bv"], dtype=np.float32)
    s2 = np.asarray(inputs["s2"], dtype=np.float32)

    nc = _get_nc()
    in_maps = []
    for core in range(8):
        b, j = core // 2, core % 2
        sl = slice(j * CPC, (j + 1) * CPC)
        in_maps.append({
            "x_b": np.ascontiguousarray(x[b]),
            "wq": np.ascontiguousarray(wq[sl]),
            "wk": np.ascontiguousarray(wk[sl]),
            "wv": np.ascontiguousarray(wv[sl]),
            "bq": np.ascontiguousarray(bq[sl]),
            "bk": np.ascontiguousarray(bk[sl]),
            "bv": np.ascontiguousarray(bv[sl]),
            "s2h": np.ascontiguousarray(s2[2 * j:2 * j + 2]),
            "x_res": np.ascontiguousarray(x[b, sl, :]),
        })
    res = run_bass_kernel_spmd(nc, in_maps, core_ids=list(range(8)))
    out = np.empty((B, F, T), dtype=np.float32)
    for core in range(8):
        b, j = core // 2, core % 2
        out[b, j * CPC:(j + 1) * CPC, :] = res.results[core]["out_c"]
    return out
